# revision 6
# baseline (speedup 1.0000x reference)
"""Trainium2 Bass kernel for nn_MultiHeadAttention_39341900431503.

8-core tensor-parallel multi-head attention (B=1, S=2048, HIDDEN=2048, 16 heads,
head_dim=128). Each core computes 2 heads end-to-end (QKV proj, RoPE, causal
attention, out-proj partial); host gathers/unpermutes and sums out-proj partials.

All matmuls run in float32r (E8M11; ~1.5e-4 rel err) at full PE rate.
"""
import os
import numpy as np
from contextlib import ExitStack

import concourse.bacc as bacc
import concourse.tile as tile
from concourse.bass_types import AP
from concourse import mybir
from concourse import bass_utils

F32 = mybir.dt.float32
F32R = mybir.dt.float32r
EXP = mybir.ActivationFunctionType.Exp

B, S, HIDDEN = 1, 2048, 2048
QKV, HEADS = 2048, 16
D = 128                      # head dim
N_CORES = 8
HPC = HEADS // N_CORES       # heads per core = 2
SCALE = D ** -0.5
NEG_INF = -1e9
NT = S // 128                # 16 s/t tiles
NCH = S // 512               # 4 512-chunks

_BUILD_CACHE = {}


def _swap_ap(base, ncols):
    """AP over base[:, 0:ncols] with 64-col blocks swapped pairwise.

    Column order: [64:128, 0:64, 192:256, 128:192, ...].
    """
    pdim = list(base.ap[0])
    nblk = ncols // 128
    return AP(base.tensor, base.offset + 64, [pdim, [128, nblk], [-64, 2], [1, 64]])


def _rep_ap(sl, nrep):
    """AP repeating the (already sliced, contiguous) [128, F] AP nrep times along free."""
    pdim = list(sl.ap[0])
    f = sl.ap[-1][1]
    return AP(sl.tensor, sl.offset, [pdim, [0, nrep], [1, f]])


def build(mask_mode):
    """mask_mode: 'causal' | 'none' | 'full'. Returns compiled Bacc module."""
    assert mask_mode in ("causal", "none", "full")
    causal = mask_mode == "causal"
    nc = bacc.Bacc("TRN2", target_bir_lowering=False, debug=False, num_devices=N_CORES)

    # ---- DRAM I/O ----
    hT_d = nc.dram_tensor("hT", [HIDDEN, S], F32, kind="ExternalInput").ap()
    wq_d = nc.dram_tensor("wqkvT", [HIDDEN, 3 * HPC * D], F32, kind="ExternalInput").ap()
    wo_d = nc.dram_tensor("woutT", [HPC * D, HIDDEN], F32, kind="ExternalInput").ap()
    cos_d = nc.dram_tensor("cosrep", [S, 2 * D], F32, kind="ExternalInput").ap()
    sin_d = nc.dram_tensor("sinsgn", [S, 2 * D], F32, kind="ExternalInput").ap()
    if causal:
        mkd_d = nc.dram_tensor("maskd", [128, 128], F32, kind="ExternalInput").ap()
        mkdT_d = nc.dram_tensor("maskdT", [128, 128], F32, kind="ExternalInput").ap()
    if mask_mode == "full":
        mk_d = nc.dram_tensor("mask", [S, S], F32, kind="ExternalInput").ap()
        mkT_d = nc.dram_tensor("maskT", [S, S], F32, kind="ExternalInput").ap()

    aw_d = nc.dram_tensor("aw", [HPC, S, S], F32, kind="ExternalOutput").ap()
    pk_d = nc.dram_tensor("pk", [S, HPC * D], F32, kind="ExternalOutput").ap()
    pv_d = nc.dram_tensor("pv", [S, HPC * D], F32, kind="ExternalOutput").ap()
    op_d = nc.dram_tensor("outp", [S, HIDDEN], F32, kind="ExternalOutput").ap()

    def ncols_of(s_tile):
        return (s_tile + 1) * 128 if causal else S

    def tmax_of(chunk):
        return chunk * 4 + 3 if causal else NT - 1

    with tile.TileContext(nc) as tc:
        with ExitStack() as octx:
            # ---- persistent residents ----
            pers = octx.enter_context(tc.tile_pool(name="pers", bufs=1))
            qT = [pers.tile([128, S], F32R, tag=f"qT{h}", name=f"qT{h}") for h in range(HPC)]
            kT = [pers.tile([128, S], F32R, tag=f"kT{h}", name=f"kT{h}") for h in range(HPC)]
            v_all = pers.tile([128, NT, HPC * D], F32R, tag="v_all")
            ctxT = [pers.tile([128, S], F32R, tag=f"ctxT{h}", name=f"ctxT{h}") for h in range(HPC)]
            wo_sb = pers.tile([128, HPC, HIDDEN], F32R, tag="wo")
            recip_all = [pers.tile([128, NT], F32, tag=f"recip{h}", name=f"recip{h}") for h in range(HPC)]
            nc.gpsimd.dma_start(
                wo_sb[:], wo_d.rearrange("(h p) f -> p h f", p=128)
            )
            if causal:
                mkd = pers.tile([128, 128], F32, tag="mkd")
                mkdT = pers.tile([128, 128], F32, tag="mkdT")
                nc.sync.dma_start(mkd[:], mkd_d[:])
                nc.sync.dma_start(mkdT[:], mkdT_d[:])
            ones_r = pers.tile([128, 1], F32R, tag="ones_r")
            ones_f = pers.tile([128, 1], F32, tag="ones_f")
            nc.vector.memset(ones_f[:], 1.0)
            nc.vector.tensor_copy(ones_r[:], ones_f[:])
            onerow_r = pers.tile([1, 128], F32R, tag="onerow_r")
            onerow_f = pers.tile([1, 128], F32, tag="onerow_f")
            nc.vector.memset(onerow_f[:], 1.0)
            nc.vector.tensor_copy(onerow_r[:], onerow_f[:])

            # ================= Phase 1: QKV + RoPE =================
            with ExitStack() as p1:
                p1sb = p1.enter_context(tc.tile_pool(name="p1sb", bufs=2))
                wpool = p1.enter_context(tc.tile_pool(name="wpool", bufs=1))
                cpool = p1.enter_context(tc.tile_pool(name="cpool", bufs=1))
                p1ps = p1.enter_context(tc.tile_pool(name="p1ps", bufs=2, space="PSUM"))
                trps = p1.enter_context(tc.tile_pool(name="trps", bufs=2, space="PSUM"))

                w_sb = wpool.tile([128, NT, 3 * HPC * D], F32R, tag="wqkv")
                nc.gpsimd.dma_start(w_sb[:], wq_d.rearrange("(a p) n -> p a n", p=128))
                cos_sb = cpool.tile([128, NT, 2 * D], F32, tag="cos")
                sin_sb = cpool.tile([128, NT, 2 * D], F32, tag="sin")
                nc.sync.dma_start(cos_sb[:], cos_d.rearrange("(a p) f -> p a f", p=128))
                nc.sync.dma_start(sin_sb[:], sin_d.rearrange("(a p) f -> p a f", p=128))
                ident = cpool.tile([128, 128], F32, tag="ident")
                from concourse import masks as _masks
                _masks.make_identity(nc, ident[:])

                QW = 2 * HPC * D       # 512: q+k region width
                VW = HPC * D           # 256: v region width

                for st in range(NT):
                    s0 = st * 128
                    h_sb = p1sb.tile([128, NT, 128], F32R, tag="hT")
                    nc.gpsimd.dma_start(
                        h_sb[:], hT_d[:, s0 : s0 + 128].rearrange("(a p) s -> p a s", p=128)
                    )
                    qkv_ps = p1ps.tile([128, 3 * HPC * D], F32, tag="qkv")
                    for a in range(NT):
                        nc.tensor.matmul(
                            qkv_ps[:, 0:512], h_sb[:, a, :], w_sb[:, a, 0:512],
                            start=(a == 0), stop=(a == NT - 1),
                        )
                        nc.tensor.matmul(
                            qkv_ps[:, 512:768], h_sb[:, a, :], w_sb[:, a, 512:768],
                            start=(a == 0), stop=(a == NT - 1),
                        )

                    # present_k (pre-RoPE) / present_v out; v also to f32r resident
                    knat = p1sb.tile([128, VW], F32, tag="knat")
                    nc.scalar.copy(knat[:], qkv_ps[:, VW : 2 * VW])
                    nc.sync.dma_start(pk_d[s0 : s0 + 128, :], knat[:])
                    vnat = p1sb.tile([128, VW], F32, tag="vnat")
                    nc.scalar.copy(vnat[:], qkv_ps[:, 2 * VW : 3 * VW])
                    nc.sync.dma_start(pv_d[s0 : s0 + 128, :], vnat[:])
                    nc.vector.tensor_copy(v_all[:, st, :], qkv_ps[:, 2 * VW : 3 * VW])

                    # RoPE on q+k region (cols 0:512):
                    # roped = x * cosrep + swap(x) * sinsgn
                    base = qkv_ps[:]
                    tA = p1sb.tile([128, QW], F32, tag="ropeA")
                    tB = p1sb.tile([128, QW], F32, tag="ropeB")
                    roped = p1sb.tile([128, QW], F32, tag="roped")
                    cos_ap = _rep_ap(cos_sb[:, st, :], 2)
                    sin_ap = _rep_ap(sin_sb[:, st, :], 2)
                    out3 = tA[:].rearrange("p (a f) -> p a f", a=2, f=2 * D)
                    nc.vector.tensor_mul(out3, base[:, 0:QW].rearrange("p (a f) -> p a f", a=2, f=2 * D), cos_ap)
                    nc.vector.tensor_mul(tB[:].rearrange("p (a f) -> p a f", a=2, f=2 * D), _swap_ap(base, QW), sin_ap)
                    nc.vector.tensor_add(roped[:], tA[:], tB[:])

                    # transpose roped q/k 128-blocks into resident qT/kT (f32r)
                    for j in range(2 * HPC):
                        t_ps = trps.tile([128, 128], F32, tag="tr")
                        nc.tensor.transpose(t_ps[:], roped[:, j * 128 : (j + 1) * 128], ident[:])
                        dst = qT[j] if j < HPC else kT[j - HPC]
                        nc.vector.tensor_copy(dst[:, s0 : s0 + 128], t_ps[:])

            # ================= Phase 2: attention =================
            with ExitStack() as p2:
                pstps = p2.enter_context(tc.tile_pool(name="pstps", bufs=2, space="PSUM"))
                ptps = p2.enter_context(tc.tile_pool(name="ptps", bufs=2, space="PSUM"))
                ctxps = p2.enter_context(tc.tile_pool(name="ctxps", bufs=2, space="PSUM"))
                srps = p2.enter_context(tc.tile_pool(name="srps", bufs=1, space="PSUM"))
                p2sb = p2.enter_context(tc.tile_pool(name="p2sb", bufs=2))
                p2sb3 = p2.enter_context(tc.tile_pool(name="p2sb3", bufs=3))
                mpool = p2.enter_context(tc.tile_pool(name="mpool", bufs=3))

                for h in range(HPC):
                    # ---- ST orientation: p[s, t] for output + row sums ----
                    for st in range(NT):
                        s0 = st * 128
                        ncols = ncols_of(st)
                        nchunk = (ncols + 511) // 512
                        pst = p2sb3.tile([128, S], F32, tag="pst")
                        sums = p2sb.tile([128, NCH], F32, tag="sums")
                        for c in range(nchunk):
                            n = min(512, ncols - c * 512)
                            pps = pstps.tile([128, 512], F32, tag="pst_ps")
                            nc.tensor.matmul(
                                pps[:, 0:n], qT[h][:, s0 : s0 + 128],
                                kT[h][:, c * 512 : c * 512 + n],
                                start=True, stop=True,
                            )
                            if causal and c == nchunk - 1:
                                off = ncols - 128 - c * 512
                                nc.vector.tensor_add(
                                    pps[:, off : off + 128], pps[:, off : off + 128], mkd[:]
                                )
                            elif mask_mode == "full":
                                mt = mpool.tile([128, 512], F32, tag="mst")
                                nc.sync.dma_start(mt[:, 0:n], mk_d[s0 : s0 + 128, c * 512 : c * 512 + n])
                                nc.vector.tensor_add(pps[:, 0:n], pps[:, 0:n], mt[:, 0:n])
                            nc.scalar.activation(
                                pst[:, c * 512 : c * 512 + n], pps[:, 0:n], EXP,
                                scale=SCALE, accum_out=sums[:, c : c + 1],
                            )
                        tot = p2sb.tile([128, 1], F32, tag="tot")
                        if nchunk > 1:
                            nc.vector.reduce_sum(tot[:], sums[:, 0:nchunk], axis=mybir.AxisListType.X)
                        else:
                            nc.vector.tensor_copy(tot[:], sums[:, 0:1])
                        nc.vector.reciprocal(recip_all[h][:, st : st + 1], tot[:])
                        pno = p2sb3.tile([128, S], F32, tag="pno")
                        nc.vector.tensor_scalar_mul(
                            pno[:, 0:ncols], pst[:, 0:ncols], recip_all[h][:, st : st + 1]
                        )
                        nc.sync.dma_start(aw_d[h, s0 : s0 + 128, 0:ncols], pno[:, 0:ncols])

                    # ---- T orientation + AV + row-sums(row layout) ----
                    for c in range(NCH):
                        tmax = tmax_of(c)
                        cps = ctxps.tile([128, 512], F32, tag="ctx")
                        srow = srps.tile([1, 512], F32, tag="srow")
                        for t in range(tmax + 1):
                            t0 = t * 128
                            # causal: columns s < t0 of this chunk are fully
                            # masked -> compute only cols [off, 512)
                            off = max(0, t0 - c * 512) if causal else 0
                            ptp = ptps.tile([128, 512], F32, tag="pt_ps")
                            nc.tensor.matmul(
                                ptp[:, off:512], kT[h][:, t0 : t0 + 128],
                                qT[h][:, c * 512 + off : (c + 1) * 512],
                                start=True, stop=True,
                            )
                            if causal and t // 4 == c:
                                nc.vector.tensor_add(
                                    ptp[:, off : off + 128], ptp[:, off : off + 128], mkdT[:]
                                )
                            elif mask_mode == "full":
                                mtT = mpool.tile([128, 512], F32, tag="mtT")
                                nc.sync.dma_start(mtT[:], mkT_d[t0 : t0 + 128, c * 512 : (c + 1) * 512])
                                nc.vector.tensor_add(ptp[:], ptp[:], mtT[:])
                            ptsb = p2sb3.tile([128, 512], F32R, tag="ptsb")
                            nc.scalar.activation(ptsb[:, off:512], ptp[:, off:512], EXP, scale=SCALE)
                            nc.tensor.matmul(
                                cps[:, off:512], v_all[:, t, h * D : (h + 1) * D], ptsb[:, off:512],
                                start=(t == 0), stop=(t == tmax), skip_group_check=True,
                            )
                            nc.tensor.matmul(
                                srow[:, off:512], ones_r[:], ptsb[:, off:512],
                                start=(t == 0), stop=(t == tmax), skip_group_check=True,
                            )
                        rrow = p2sb.tile([1, 512], F32R, tag="rrow")
                        with nc.allow_low_precision(reason="f32r reciprocal row for ctx normalize"):
                            nc.vector.reciprocal(rrow[:], srow[:])
                        rbc_ps = srps.tile([128, 512], F32, tag="rbc_ps")
                        nc.tensor.matmul(rbc_ps[:], onerow_r[:], rrow[:], start=True, stop=True)
                        rbc = p2sb.tile([128, 512], F32, tag="rbc")
                        nc.vector.tensor_copy(rbc[:], rbc_ps[:])
                        nc.vector.tensor_mul(ctxT[h][:, c * 512 : (c + 1) * 512], cps[:], rbc[:])

            # ================= Phase 3: out projection =================
            with ExitStack() as p3:
                ops_ = p3.enter_context(tc.tile_pool(name="ops", bufs=3, space="PSUM"))
                osb_p = p3.enter_context(tc.tile_pool(name="osbp", bufs=2))
                for st in range(NT):
                    s0 = st * 128
                    osb = osb_p.tile([128, HIDDEN], F32, tag="osb")
                    for c in range(NCH):
                        o_ps = ops_.tile([128, 512], F32, tag="o_ps")
                        for h in range(HPC):
                            nc.tensor.matmul(
                                o_ps[:], ctxT[h][:, s0 : s0 + 128],
                                wo_sb[:, h, c * 512 : (c + 1) * 512],
                                start=(h == 0), stop=(h == HPC - 1),
                                skip_group_check=True,
                            )
                        nc.vector.tensor_copy(osb[:, c * 512 : (c + 1) * 512], o_ps[:])
                    nc.sync.dma_start(op_d[s0 : s0 + 128, :], osb[:])

    nc.compile()
    return nc


def _host_consts():
    inv_freq = 1.0 / (10000.0 ** (np.arange(0, D, 2, dtype=np.float64) / D))  # [64]
    ang = np.arange(S, dtype=np.float64)[:, None] * inv_freq[None, :]          # [S, 64]
    cos = np.cos(ang).astype(np.float32)
    sin = np.sin(ang).astype(np.float32)
    cosrep = np.tile(cos, (1, 4))                                              # [S, 256]
    sinsgn = np.concatenate([-sin, sin, -sin, sin], axis=1)                    # [S, 256]
    i = np.arange(128)
    maskd = np.where(i[:, None] >= i[None, :], 0.0, NEG_INF).astype(np.float32)
    maskdT = maskd.T.copy()
    return cosrep, sinsgn, maskd, maskdT


def _detect_mode(attn_mask):
    if not np.any(attn_mask):
        return "none"
    i = np.arange(S)
    causal_ref = np.where(i[:, None] >= i[None, :], 0.0, np.float32(NEG_INF)).astype(np.float32)
    if np.array_equal(attn_mask, causal_ref):
        return "causal"
    return "full"


def kernel(hidden_states, attn_mask, w_qkv, w_out):
    hidden_states = np.ascontiguousarray(hidden_states, dtype=np.float32)
    attn_mask = np.ascontiguousarray(attn_mask, dtype=np.float32)
    w_qkv = np.ascontiguousarray(w_qkv, dtype=np.float32)
    w_out = np.ascontiguousarray(w_out, dtype=np.float32)

    mode = _detect_mode(attn_mask)
    if mode not in _BUILD_CACHE:
        _BUILD_CACHE[mode] = build(mode)
    nc = _BUILD_CACHE[mode]

    cosrep, sinsgn, maskd, maskdT = _host_consts()
    hT = np.ascontiguousarray(hidden_states[0].T)          # [HIDDEN, S]

    in_maps = []
    for core in range(N_CORES):
        heads = [HPC * core + j for j in range(HPC)]
        d = np.arange(D)
        qk_rows = []
        for base in (0, QKV, 2 * QKV):                     # q, k, v
            for h in heads:
                qk_rows.append(base + d * HEADS + h)
        rows = np.concatenate(qk_rows)
        wqkvT = np.ascontiguousarray(w_qkv[rows, :].T)     # [HIDDEN, 768]
        cols = np.concatenate([d * HEADS + h for h in heads])
        woutT = np.ascontiguousarray(w_out[:, cols].T)     # [256, HIDDEN]
        m = {
            "hT": hT, "wqkvT": wqkvT, "woutT": woutT,
            "cosrep": cosrep, "sinsgn": sinsgn,
        }
        if mode == "causal":
            m["maskd"] = maskd
            m["maskdT"] = maskdT
        if mode == "full":
            m["mask"] = attn_mask
            m["maskT"] = np.ascontiguousarray(attn_mask.T)
        in_maps.append(m)

    trace = bool(int(os.environ.get("KERNEL_TRACE", "0")))
    kwargs = {}
    if trace:
        import ntff_shim
        ntff_shim.install()
        kwargs = {"trace": True, "trace_cores": [0]}
    res = bass_utils.run_bass_kernel_spmd(nc, in_maps, core_ids=list(range(N_CORES)), **kwargs)
    kernel.last_exec_time_ns = res.exec_time_ns

    attn_output = np.zeros((S, HIDDEN), dtype=np.float32)
    aw = np.empty((HEADS, S, S), dtype=np.float32)
    pk = np.empty((HEADS, S, D), dtype=np.float32)
    pv = np.empty((HEADS, S, D), dtype=np.float32)
    for core in range(N_CORES):
        r = res.results[core]
        attn_output += r["outp"]
        aw[HPC * core : HPC * (core + 1)] = r["aw"]
        pk[HPC * core : HPC * (core + 1)] = r["pk"].reshape(S, HPC, D).transpose(1, 0, 2)
        pv[HPC * core : HPC * (core + 1)] = r["pv"].reshape(S, HPC, D).transpose(1, 0, 2)

    return (
        attn_output[None, :, :],
        aw[None, :, :, :],
        pk[None, :, :, :],
        pv[None, :, :, :],
    )


# revision 9
# speedup vs baseline: 1.1720x; 1.1720x over previous
"""Trainium2 Bass kernel for nn_MultiHeadAttention_39341900431503.

8-core tensor-parallel multi-head attention (B=1, S=2048, HIDDEN=2048, 16 heads,
head_dim=128). Each core computes 2 heads end-to-end (QKV proj, RoPE, causal
attention, out-proj partial); host gathers/unpermutes and sums out-proj partials.

All matmuls run in float32r (E8M11; ~1.5e-4 rel err) at full PE rate.
"""
import os
import numpy as np
from contextlib import ExitStack

import concourse.bacc as bacc
import concourse.tile as tile
from concourse.bass_types import AP
from concourse import mybir
from concourse import bass_utils

F32 = mybir.dt.float32
F32R = mybir.dt.float32r
EXP = mybir.ActivationFunctionType.Exp

B, S, HIDDEN = 1, 2048, 2048
QKV, HEADS = 2048, 16
D = 128                      # head dim
N_CORES = 8
HPC = HEADS // N_CORES       # heads per core = 2
SCALE = D ** -0.5
NEG_INF = -1e9
NT = S // 128                # 16 s/t tiles
NCH = S // 512               # 4 512-chunks

_BUILD_CACHE = {}


def _swap_ap(base, ncols):
    """AP over base[:, 0:ncols] with 64-col blocks swapped pairwise."""
    pdim = list(base.ap[0])
    nblk = ncols // 128
    return AP(base.tensor, base.offset + 64, [pdim, [128, nblk], [-64, 2], [1, 64]])


def _rep_ap(sl, nrep):
    """AP repeating a contiguous [128, F] slice nrep times along free."""
    pdim = list(sl.ap[0])
    f = sl.ap[-1][1]
    return AP(sl.tensor, sl.offset, [pdim, [0, nrep], [1, f]])


def build(mask_mode):
    """mask_mode: 'causal' | 'none' | 'full'. Returns compiled Bacc module."""
    assert mask_mode in ("causal", "none", "full")
    causal = mask_mode == "causal"
    nc = bacc.Bacc("TRN2", target_bir_lowering=False, debug=False, num_devices=N_CORES)

    # ---- DRAM I/O ----
    hT_d = nc.dram_tensor("hT", [HIDDEN, S], F32, kind="ExternalInput").ap()
    wq_d = nc.dram_tensor("wqkvT", [HIDDEN, 3 * HPC * D], F32, kind="ExternalInput").ap()
    wo_d = nc.dram_tensor("woutT", [HPC * D, HIDDEN], F32, kind="ExternalInput").ap()
    cos_d = nc.dram_tensor("cosrep", [S, 2 * D], F32, kind="ExternalInput").ap()
    sin_d = nc.dram_tensor("sinsgn", [S, 2 * D], F32, kind="ExternalInput").ap()
    sel_d = nc.dram_tensor("selmat", [NT, NT * 128], F32, kind="ExternalInput").ap()
    if causal:
        mkd_d = nc.dram_tensor("maskd", [128, 128], F32, kind="ExternalInput").ap()
        mkdT_d = nc.dram_tensor("maskdT", [128, 128], F32, kind="ExternalInput").ap()
    if mask_mode == "full":
        mk_d = nc.dram_tensor("mask", [S, S], F32, kind="ExternalInput").ap()
        mkT_d = nc.dram_tensor("maskT", [S, S], F32, kind="ExternalInput").ap()

    aw_d = nc.dram_tensor("aw", [HPC, S, S], F32, kind="ExternalOutput").ap()
    pk_d = nc.dram_tensor("pk", [S, HPC * D], F32, kind="ExternalOutput").ap()
    pv_d = nc.dram_tensor("pv", [S, HPC * D], F32, kind="ExternalOutput").ap()
    # out-proj partial, TRANSPOSED: [hid, s]; host sums cores then transposes
    op_d = nc.dram_tensor("outp", [HIDDEN, S], F32, kind="ExternalOutput").ap()

    def ncols_of(s_tile):
        return (s_tile + 1) * 128 if causal else S

    with tile.TileContext(nc) as tc:
        with ExitStack() as octx:
            # ---- persistent residents ----
            pers = octx.enter_context(tc.tile_pool(name="pers", bufs=1))
            qT = [pers.tile([128, S], F32R, tag=f"qT{h}", name=f"qT{h}") for h in range(HPC)]
            kT = [pers.tile([128, S], F32R, tag=f"kT{h}", name=f"kT{h}") for h in range(HPC)]
            v_all = pers.tile([128, NT, HPC * D], F32R, tag="v_all")
            ctxT = [pers.tile([128, S], F32R, tag=f"ctxT{h}", name=f"ctxT{h}") for h in range(HPC)]
            wo_sb = pers.tile([128, HPC, HIDDEN], F32R, tag="wo")
            recip_all = [pers.tile([128, NT], F32, tag=f"recip{h}", name=f"recip{h}") for h in range(HPC)]
            sel_sb = pers.tile([16, NT * 128], F32R, tag="sel")
            ident = pers.tile([128, 128], F32, tag="ident")
            nc.gpsimd.dma_start(wo_sb[:], wo_d.rearrange("(h p) f -> p h f", p=128))
            nc.gpsimd.dma_start(sel_sb[:], sel_d[:])
            from concourse import masks as _masks
            _masks.make_identity(nc, ident[:])
            if causal:
                mkd = pers.tile([128, 128], F32, tag="mkd")
                mkdT = pers.tile([128, 128], F32, tag="mkdT")
                nc.sync.dma_start(mkd[:], mkd_d[:])
                nc.sync.dma_start(mkdT[:], mkdT_d[:])

            # ================= Phase 1: QKV + RoPE =================
            with ExitStack() as p1:
                p1sb = p1.enter_context(tc.tile_pool(name="p1sb", bufs=2))
                wpool = p1.enter_context(tc.tile_pool(name="wpool", bufs=1))
                cpool = p1.enter_context(tc.tile_pool(name="cpool", bufs=1))
                p1ps = p1.enter_context(tc.tile_pool(name="p1ps", bufs=2, space="PSUM"))
                trps = p1.enter_context(tc.tile_pool(name="trps", bufs=2, space="PSUM"))

                w_sb = wpool.tile([128, NT, 3 * HPC * D], F32R, tag="wqkv")
                nc.gpsimd.dma_start(w_sb[:], wq_d.rearrange("(a p) n -> p a n", p=128))
                cos_sb = cpool.tile([128, NT, 2 * D], F32, tag="cos")
                sin_sb = cpool.tile([128, NT, 2 * D], F32, tag="sin")
                nc.sync.dma_start(cos_sb[:], cos_d.rearrange("(a p) f -> p a f", p=128))
                nc.sync.dma_start(sin_sb[:], sin_d.rearrange("(a p) f -> p a f", p=128))

                QW = 2 * HPC * D       # 512: q+k region width
                VW = HPC * D           # 256: v region width

                for st in range(NT):
                    s0 = st * 128
                    h_sb = p1sb.tile([128, NT, 128], F32R, tag="hT")
                    nc.gpsimd.dma_start(
                        h_sb[:], hT_d[:, s0 : s0 + 128].rearrange("(a p) s -> p a s", p=128)
                    )
                    qkv_ps = p1ps.tile([128, 3 * HPC * D], F32, tag="qkv")
                    for a in range(NT):
                        nc.tensor.matmul(
                            qkv_ps[:, 0:512], h_sb[:, a, :], w_sb[:, a, 0:512],
                            start=(a == 0), stop=(a == NT - 1),
                        )
                        nc.tensor.matmul(
                            qkv_ps[:, 512:768], h_sb[:, a, :], w_sb[:, a, 512:768],
                            start=(a == 0), stop=(a == NT - 1),
                        )

                    # present_k (pre-RoPE) / present_v out; v also to f32r resident
                    knat = p1sb.tile([128, VW], F32, tag="knat")
                    nc.scalar.copy(knat[:], qkv_ps[:, VW : 2 * VW])
                    nc.sync.dma_start(pk_d[s0 : s0 + 128, :], knat[:])
                    vnat = p1sb.tile([128, VW], F32, tag="vnat")
                    nc.scalar.copy(vnat[:], qkv_ps[:, 2 * VW : 3 * VW])
                    nc.sync.dma_start(pv_d[s0 : s0 + 128, :], vnat[:])
                    nc.vector.tensor_copy(v_all[:, st, :], qkv_ps[:, 2 * VW : 3 * VW])

                    # RoPE on q+k region: roped = x*cosrep + swap(x)*sinsgn
                    base = qkv_ps[:]
                    tA = p1sb.tile([128, QW], F32, tag="ropeA")
                    tB = p1sb.tile([128, QW], F32, tag="ropeB")
                    roped = p1sb.tile([128, QW], F32, tag="roped")
                    cos_ap = _rep_ap(cos_sb[:, st, :], 2)
                    sin_ap = _rep_ap(sin_sb[:, st, :], 2)
                    nc.vector.tensor_mul(
                        tA[:].rearrange("p (a f) -> p a f", a=2, f=2 * D),
                        base[:, 0:QW].rearrange("p (a f) -> p a f", a=2, f=2 * D), cos_ap)
                    nc.vector.tensor_mul(
                        tB[:].rearrange("p (a f) -> p a f", a=2, f=2 * D),
                        _swap_ap(base, QW), sin_ap)
                    nc.vector.tensor_add(roped[:], tA[:], tB[:])

                    # transpose roped q/k 128-blocks into resident qT/kT (f32r)
                    for j in range(2 * HPC):
                        t_ps = trps.tile([128, 128], F32, tag="tr")
                        nc.tensor.transpose(t_ps[:], roped[:, j * 128 : (j + 1) * 128], ident[:])
                        dst = qT[j] if j < HPC else kT[j - HPC]
                        nc.vector.tensor_copy(dst[:, s0 : s0 + 128], t_ps[:])

            # ================= Phase 2: attention =================
            with ExitStack() as p2:
                p512 = p2.enter_context(tc.tile_pool(name="p512", bufs=3, space="PSUM"))
                ctxps = p2.enter_context(tc.tile_pool(name="ctxps", bufs=1, space="PSUM"))
                p2sb = p2.enter_context(tc.tile_pool(name="p2sb", bufs=2))
                p2sb3 = p2.enter_context(tc.tile_pool(name="p2sb3", bufs=4))
                mpool = p2.enter_context(tc.tile_pool(name="mpool", bufs=3))

                for h in range(HPC):
                    # ---- ST orientation: p[s, t] for aw output + row sums ----
                    for st in range(NT):
                        s0 = st * 128
                        ncols = ncols_of(st)
                        nchunk = (ncols + 511) // 512
                        pst = p2sb3.tile([128, S], F32, tag="pst")
                        sums = p2sb.tile([128, NCH], F32, tag="sums")
                        for c in range(nchunk):
                            n = min(512, ncols - c * 512)
                            pps = p512.tile([128, 512], F32, tag="p512", name=f"pps{h}_{st}_{c}")
                            nc.tensor.matmul(
                                pps[:, 0:n], qT[h][:, s0 : s0 + 128],
                                kT[h][:, c * 512 : c * 512 + n],
                                start=True, stop=True,
                            )
                            if causal and c == nchunk - 1:
                                off = ncols - 128 - c * 512
                                nc.vector.tensor_add(
                                    pps[:, off : off + 128], pps[:, off : off + 128], mkd[:]
                                )
                            elif mask_mode == "full":
                                mt = mpool.tile([128, 512], F32, tag="mst")
                                nc.sync.dma_start(mt[:, 0:n], mk_d[s0 : s0 + 128, c * 512 : c * 512 + n])
                                nc.vector.tensor_add(pps[:, 0:n], pps[:, 0:n], mt[:, 0:n])
                            nc.scalar.activation(
                                pst[:, c * 512 : c * 512 + n], pps[:, 0:n], EXP,
                                scale=SCALE, accum_out=sums[:, c : c + 1],
                            )
                        tot = p2sb.tile([128, 1], F32, tag="tot")
                        if nchunk > 1:
                            nc.vector.reduce_sum(tot[:], sums[:, 0:nchunk], axis=mybir.AxisListType.X)
                        else:
                            nc.vector.tensor_copy(tot[:], sums[:, 0:1])
                        nc.vector.reciprocal(recip_all[h][:, st : st + 1], tot[:])
                        pno = p2sb3.tile([128, S], F32, tag="pno")
                        nc.vector.tensor_scalar_mul(
                            pno[:, 0:ncols], pst[:, 0:ncols], recip_all[h][:, st : st + 1]
                        )
                        nc.sync.dma_start(aw_d[h, s0 : s0 + 128, 0:ncols], pno[:, 0:ncols])

                    # ---- row-layout recip broadcast:
                    # rbc[p, s] = 1/rowsum[s], via PE transpose + select-matmuls
                    rt_ps = p512.tile([16, 128], F32, tag="p512", name=f"rtps{h}")
                    nc.tensor.transpose(rt_ps[:], recip_all[h][:], ident[:])
                    rt16 = p2sb.tile([16, 128], F32R, tag="rt16")
                    nc.vector.tensor_copy(rt16[:], rt_ps[:])
                    rbc = p2sb.tile([128, S], F32, tag="rbc")
                    for st in range(NT):
                        rb_ps = p512.tile([128, 128], F32, tag="p512", name=f"rbps{h}_{st}")
                        nc.tensor.matmul(
                            rb_ps[:], sel_sb[:, st * 128 : (st + 1) * 128], rt16[:],
                            start=True, stop=True,
                        )
                        nc.vector.tensor_copy(rbc[:, st * 128 : (st + 1) * 128], rb_ps[:])

                    # ---- T orientation + AV, t-outer to amortize stationaries ----
                    tmax_of = (lambda c: c * 4 + 3) if causal else (lambda c: NT - 1)
                    cps = [
                        ctxps.tile([128, 512], F32, tag=f"ctx{c}", name=f"ctx{h}_{c}")
                        for c in range(NCH)
                    ]
                    for t in range(NT):
                        t0 = t * 128
                        chunks = [c for c in range(NCH) if (not causal) or t <= tmax_of(c)]
                        ptps, offs = {}, {}
                        for c in chunks:
                            off = max(0, t0 - c * 512) if causal else 0
                            offs[c] = off
                            ptp = p512.tile([128, 512], F32, tag="p512", name=f"ptp{h}_{t}_{c}")
                            ptps[c] = ptp
                            nc.tensor.matmul(
                                ptp[:, off:512], kT[h][:, t0 : t0 + 128],
                                qT[h][:, c * 512 + off : (c + 1) * 512],
                                start=True, stop=True,
                            )
                        for c in chunks:
                            off = offs[c]
                            ptp = ptps[c]
                            if causal and t // 4 == c:
                                nc.vector.tensor_add(
                                    ptp[:, off : off + 128], ptp[:, off : off + 128], mkdT[:]
                                )
                            elif mask_mode == "full":
                                mtT = mpool.tile([128, 512], F32, tag="mtT")
                                nc.sync.dma_start(mtT[:], mkT_d[t0 : t0 + 128, c * 512 : (c + 1) * 512])
                                nc.vector.tensor_add(ptp[:], ptp[:], mtT[:])
                            ptsb = p2sb3.tile([128, 512], F32R, tag="ptsb", name=f"ptsb{h}_{t}_{c}")
                            nc.scalar.activation(ptsb[:, off:512], ptp[:, off:512], EXP, scale=SCALE)
                            ptps[c] = ptsb
                        for c in chunks:
                            off = offs[c]
                            nc.tensor.matmul(
                                cps[c][:, off:512], v_all[:, t, h * D : (h + 1) * D],
                                ptps[c][:, off:512],
                                start=(t == 0), stop=(t == tmax_of(c)),
                                skip_group_check=True,
                            )
                    for c in range(NCH):
                        nc.vector.tensor_mul(
                            ctxT[h][:, c * 512 : (c + 1) * 512], cps[c][:],
                            rbc[:, c * 512 : (c + 1) * 512],
                        )

            # ================= Phase 3: out projection (transposed) =================
            # outp[hid, s] = sum_d woutT[d, hid] * ctxT[d, s], accumulated over heads
            with ExitStack() as p3:
                ops_ = p3.enter_context(tc.tile_pool(name="ops", bufs=2, space="PSUM"))
                osb_p = p3.enter_context(tc.tile_pool(name="osbp", bufs=2))
                for ht in range(NT):
                    hh0 = ht * 128
                    osb = osb_p.tile([128, S], F32, tag="osb")
                    o_ps = [
                        ops_.tile([128, 512], F32, tag=f"o{c}", name=f"o{ht}_{c}")
                        for c in range(NCH)
                    ]
                    for h in range(HPC):
                        for c in range(NCH):
                            nc.tensor.matmul(
                                o_ps[c][:], wo_sb[:, h, hh0 : hh0 + 128],
                                ctxT[h][:, c * 512 : (c + 1) * 512],
                                start=(h == 0), stop=(h == HPC - 1),
                                skip_group_check=True,
                            )
                    for c in range(NCH):
                        nc.any.tensor_copy(osb[:, c * 512 : (c + 1) * 512], o_ps[c][:])
                    nc.sync.dma_start(op_d[hh0 : hh0 + 128, :], osb[:])

    nc.compile()
    return nc


def _host_consts():
    inv_freq = 1.0 / (10000.0 ** (np.arange(0, D, 2, dtype=np.float64) / D))  # [64]
    ang = np.arange(S, dtype=np.float64)[:, None] * inv_freq[None, :]          # [S, 64]
    cos = np.cos(ang).astype(np.float32)
    sin = np.sin(ang).astype(np.float32)
    cosrep = np.tile(cos, (1, 4))                                              # [S, 256]
    sinsgn = np.concatenate([-sin, sin, -sin, sin], axis=1)                    # [S, 256]
    i = np.arange(128)
    maskd = np.where(i[:, None] >= i[None, :], 0.0, NEG_INF).astype(np.float32)
    maskdT = maskd.T.copy()
    selmat = np.zeros((NT, NT * 128), dtype=np.float32)
    for st in range(NT):
        selmat[st, st * 128 : (st + 1) * 128] = 1.0
    return cosrep, sinsgn, maskd, maskdT, selmat


def _detect_mode(attn_mask):
    if not np.any(attn_mask):
        return "none"
    i = np.arange(S)
    causal_ref = np.where(i[:, None] >= i[None, :], 0.0, np.float32(NEG_INF)).astype(np.float32)
    if np.array_equal(attn_mask, causal_ref):
        return "causal"
    return "full"


def kernel(hidden_states, attn_mask, w_qkv, w_out):
    hidden_states = np.ascontiguousarray(hidden_states, dtype=np.float32)
    attn_mask = np.ascontiguousarray(attn_mask, dtype=np.float32)
    w_qkv = np.ascontiguousarray(w_qkv, dtype=np.float32)
    w_out = np.ascontiguousarray(w_out, dtype=np.float32)

    mode = _detect_mode(attn_mask)
    if mode not in _BUILD_CACHE:
        _BUILD_CACHE[mode] = build(mode)
    nc = _BUILD_CACHE[mode]

    cosrep, sinsgn, maskd, maskdT, selmat = _host_consts()
    hT = np.ascontiguousarray(hidden_states[0].T)          # [HIDDEN, S]

    in_maps = []
    for core in range(N_CORES):
        heads = [HPC * core + j for j in range(HPC)]
        d = np.arange(D)
        rows = np.concatenate(
            [base + d * HEADS + h for base in (0, QKV, 2 * QKV) for h in heads]
        )
        wqkvT = np.ascontiguousarray(w_qkv[rows, :].T)     # [HIDDEN, 768]
        cols = np.concatenate([d * HEADS + h for h in heads])
        woutT = np.ascontiguousarray(w_out[:, cols].T)     # [256, HIDDEN]
        m = {
            "hT": hT, "wqkvT": wqkvT, "woutT": woutT,
            "cosrep": cosrep, "sinsgn": sinsgn, "selmat": selmat,
        }
        if mode == "causal":
            m["maskd"] = maskd
            m["maskdT"] = maskdT
        if mode == "full":
            m["mask"] = attn_mask
            m["maskT"] = np.ascontiguousarray(attn_mask.T)
        in_maps.append(m)

    trace = bool(int(os.environ.get("KERNEL_TRACE", "0")))
    kwargs = {}
    if trace:
        import ntff_shim
        ntff_shim.install()
        kwargs = {"trace": True, "trace_cores": [0]}
    res = bass_utils.run_bass_kernel_spmd(nc, in_maps, core_ids=list(range(N_CORES)), **kwargs)
    kernel.last_exec_time_ns = res.exec_time_ns
    kernel.last_results = res

    attn_output_T = np.zeros((HIDDEN, S), dtype=np.float32)
    aw = np.empty((HEADS, S, S), dtype=np.float32)
    pk = np.empty((HEADS, S, D), dtype=np.float32)
    pv = np.empty((HEADS, S, D), dtype=np.float32)
    for core in range(N_CORES):
        r = res.results[core]
        attn_output_T += r["outp"]
        aw[HPC * core : HPC * (core + 1)] = r["aw"]
        pk[HPC * core : HPC * (core + 1)] = r["pk"].reshape(S, HPC, D).transpose(1, 0, 2)
        pv[HPC * core : HPC * (core + 1)] = r["pv"].reshape(S, HPC, D).transpose(1, 0, 2)

    return (
        np.ascontiguousarray(attn_output_T.T)[None, :, :],
        aw[None, :, :, :],
        pk[None, :, :, :],
        pv[None, :, :, :],
    )


# revision 13
# speedup vs baseline: 1.3106x; 1.1182x over previous
"""Trainium2 Bass kernel for nn_MultiHeadAttention_39341900431503.

8-core tensor-parallel multi-head attention (B=1, S=2048, HIDDEN=2048, 16 heads,
head_dim=128). Each core computes 2 heads end-to-end (QKV proj, RoPE, causal
attention, out-proj partial); host gathers/unpermutes and sums out-proj partials.

All matmuls run in float32r (E8M11; ~1.5e-4 rel err) at full PE rate.
"""
import os
import numpy as np
from contextlib import ExitStack

import concourse.bacc as bacc
import concourse.tile as tile
from concourse.bass_types import AP
from concourse import mybir
from concourse import bass_utils

F32 = mybir.dt.float32
F32R = mybir.dt.float32r
EXP = mybir.ActivationFunctionType.Exp

B, S, HIDDEN = 1, 2048, 2048
QKV, HEADS = 2048, 16
D = 128                      # head dim
N_CORES = 8
HPC = HEADS // N_CORES       # heads per core = 2
SCALE = D ** -0.5
NEG_INF = -1e9
NT = S // 128                # 16 s/t tiles
NCH = S // 512               # 4 512-chunks

_BUILD_CACHE = {}


def _swap_ap(base, ncols):
    """AP over base[:, 0:ncols] with 64-col blocks swapped pairwise."""
    pdim = list(base.ap[0])
    nblk = ncols // 128
    return AP(base.tensor, base.offset + 64, [pdim, [128, nblk], [-64, 2], [1, 64]])


def _rep_ap(sl, nrep):
    """AP repeating a contiguous [128, F] slice nrep times along free."""
    pdim = list(sl.ap[0])
    f = sl.ap[-1][1]
    return AP(sl.tensor, sl.offset, [pdim, [0, nrep], [1, f]])


def build(mask_mode):
    """mask_mode: 'causal' | 'none' | 'full'. Returns compiled Bacc module."""
    assert mask_mode in ("causal", "none", "full")
    causal = mask_mode == "causal"
    nc = bacc.Bacc("TRN2", target_bir_lowering=False, debug=False, num_devices=N_CORES)

    # ---- DRAM I/O ----
    hT_d = nc.dram_tensor("hT", [HIDDEN, S], F32, kind="ExternalInput").ap()
    wq_d = nc.dram_tensor("wqkvT", [HIDDEN, 3 * HPC * D], F32, kind="ExternalInput").ap()
    wo_d = nc.dram_tensor("woutT", [HPC * D, HIDDEN], F32, kind="ExternalInput").ap()
    cos_d = nc.dram_tensor("cosrep", [S, 2 * D], F32, kind="ExternalInput").ap()
    sin_d = nc.dram_tensor("sinsgn", [S, 2 * D], F32, kind="ExternalInput").ap()
    sel_d = nc.dram_tensor("selmat", [NT, NT * 128], F32, kind="ExternalInput").ap()
    if causal:
        mkd_d = nc.dram_tensor("maskd", [128, 128], F32, kind="ExternalInput").ap()
        mkdT_d = nc.dram_tensor("maskdT", [128, 128], F32, kind="ExternalInput").ap()
    if mask_mode == "full":
        mk_d = nc.dram_tensor("mask", [S, S], F32, kind="ExternalInput").ap()
        mkT_d = nc.dram_tensor("maskT", [S, S], F32, kind="ExternalInput").ap()

    aw_d = nc.dram_tensor("aw", [HPC, S, S], F32, kind="ExternalOutput").ap()
    pk_d = nc.dram_tensor("pk", [S, HPC * D], F32, kind="ExternalOutput").ap()
    pv_d = nc.dram_tensor("pv", [S, HPC * D], F32, kind="ExternalOutput").ap()
    # out-proj partial, TRANSPOSED: [hid, s]; host sums cores then transposes
    op_d = nc.dram_tensor("outp", [HIDDEN, S], F32, kind="ExternalOutput").ap()

    def ncols_of(s_tile):
        return (s_tile + 1) * 128 if causal else S

    with tile.TileContext(nc) as tc:
        with ExitStack() as octx:
            # ---- persistent residents ----
            pers = octx.enter_context(tc.tile_pool(name="pers", bufs=1))
            qT = [pers.tile([128, S], F32R, tag=f"qT{h}", name=f"qT{h}") for h in range(HPC)]
            kT = [pers.tile([128, S], F32R, tag=f"kT{h}", name=f"kT{h}") for h in range(HPC)]
            v_all = pers.tile([128, NT, HPC * D], F32R, tag="v_all")
            ctxT = [pers.tile([128, S], F32R, tag=f"ctxT{h}", name=f"ctxT{h}") for h in range(HPC)]
            wo_sb = pers.tile([128, HPC, HIDDEN], F32R, tag="wo")
            recip_all = [pers.tile([128, NT], F32, tag=f"recip{h}", name=f"recip{h}") for h in range(HPC)]
            sel_sb = pers.tile([16, NT * 128], F32R, tag="sel")
            ident = pers.tile([128, 128], F32, tag="ident")
            nc.gpsimd.dma_start(wo_sb[:], wo_d.rearrange("(h p) f -> p h f", p=128))
            nc.gpsimd.dma_start(sel_sb[:], sel_d[:])
            from concourse import masks as _masks
            _masks.make_identity(nc, ident[:])
            if causal:
                mkd = pers.tile([128, 128], F32, tag="mkd")
                mkdT = pers.tile([128, 128], F32, tag="mkdT")
                nc.sync.dma_start(mkd[:], mkd_d[:])
                nc.sync.dma_start(mkdT[:], mkdT_d[:])

            # ================= Phase 1: QKV + RoPE =================
            with ExitStack() as p1:
                p1sb = p1.enter_context(tc.tile_pool(name="p1sb", bufs=2))
                wpool = p1.enter_context(tc.tile_pool(name="wpool", bufs=1))
                p1ps = p1.enter_context(tc.tile_pool(name="p1ps", bufs=2, space="PSUM"))
                trps = p1.enter_context(tc.tile_pool(name="trps", bufs=2, space="PSUM"))

                w_sb = wpool.tile([128, NT, 3 * HPC * D], F32R, tag="wqkv")
                nc.gpsimd.dma_start(w_sb[:], wq_d.rearrange("(a p) n -> p a n", p=128))


                QW = 2 * HPC * D       # 512: q+k region width
                VW = HPC * D           # 256: v region width

                for st in range(NT):
                    s0 = st * 128
                    h_sb = p1sb.tile([128, NT, 128], F32R, tag="hT", bufs=3)
                    nc.gpsimd.dma_start(
                        h_sb[:], hT_d[:, s0 : s0 + 128].rearrange("(a p) s -> p a s", p=128)
                    )
                    qkv_ps = p1ps.tile([128, 3 * HPC * D], F32, tag="qkv")
                    for a in range(NT):
                        nc.tensor.matmul(
                            qkv_ps[:, 0:512], h_sb[:, a, :], w_sb[:, a, 0:512],
                            start=(a == 0), stop=(a == NT - 1),
                        )
                        nc.tensor.matmul(
                            qkv_ps[:, 512:768], h_sb[:, a, :], w_sb[:, a, 512:768],
                            start=(a == 0), stop=(a == NT - 1),
                        )

                    # present_k (pre-RoPE) / present_v out; v also to f32r resident
                    knat = p1sb.tile([128, VW], F32, tag="knat")
                    nc.any.tensor_copy(knat[:], qkv_ps[:, VW : 2 * VW])
                    nc.sync.dma_start(pk_d[s0 : s0 + 128, :], knat[:])
                    vnat = p1sb.tile([128, VW], F32, tag="vnat")
                    nc.any.tensor_copy(vnat[:], qkv_ps[:, 2 * VW : 3 * VW])
                    nc.sync.dma_start(pv_d[s0 : s0 + 128, :], vnat[:])
                    nc.vector.tensor_copy(v_all[:, st, :], qkv_ps[:, 2 * VW : 3 * VW])

                    # RoPE on q+k region: roped = x*cosrep + swap(x)*sinsgn
                    base = qkv_ps[:]
                    tA = p1sb.tile([128, QW], F32, tag="ropeA")
                    tB = p1sb.tile([128, QW], F32, tag="ropeB")
                    roped = p1sb.tile([128, QW], F32, tag="roped")
                    cos_sb = p1sb.tile([128, 2 * D], F32, tag="cos")
                    sin_sb = p1sb.tile([128, 2 * D], F32, tag="sin")
                    nc.sync.dma_start(cos_sb[:], cos_d[s0 : s0 + 128, :])
                    nc.sync.dma_start(sin_sb[:], sin_d[s0 : s0 + 128, :])
                    cos_ap = _rep_ap(cos_sb[:], 2)
                    sin_ap = _rep_ap(sin_sb[:], 2)
                    nc.vector.tensor_mul(
                        tA[:].rearrange("p (a f) -> p a f", a=2, f=2 * D),
                        base[:, 0:QW].rearrange("p (a f) -> p a f", a=2, f=2 * D), cos_ap)
                    nc.vector.tensor_mul(
                        tB[:].rearrange("p (a f) -> p a f", a=2, f=2 * D),
                        _swap_ap(base, QW), sin_ap)
                    nc.vector.tensor_add(roped[:], tA[:], tB[:])

                    # transpose roped q/k 128-blocks into resident qT/kT (f32r)
                    for j in range(2 * HPC):
                        t_ps = trps.tile([128, 128], F32, tag="tr")
                        nc.tensor.transpose(t_ps[:], roped[:, j * 128 : (j + 1) * 128], ident[:])
                        dst = qT[j] if j < HPC else kT[j - HPC]
                        nc.vector.tensor_copy(dst[:, s0 : s0 + 128], t_ps[:])

            # ================= Phase 2: attention =================
            with ExitStack() as p2:
                p512 = p2.enter_context(tc.tile_pool(name="p512", bufs=4, space="PSUM"))
                ctxps = p2.enter_context(tc.tile_pool(name="ctxps", bufs=1, space="PSUM"))
                p2sb = p2.enter_context(tc.tile_pool(name="p2sb", bufs=2))
                p2sb3 = p2.enter_context(tc.tile_pool(name="p2sb3", bufs=3))
                ptpool = p2.enter_context(tc.tile_pool(name="ptpool", bufs=10))
                mpool = p2.enter_context(tc.tile_pool(name="mpool", bufs=3))

                for h in range(HPC):
                    # ---- ST orientation: p[s, t] for aw output + row sums ----
                    for st in range(NT):
                        s0 = st * 128
                        ncols = ncols_of(st)
                        nchunk = (ncols + 511) // 512
                        pst = p2sb3.tile([128, S], F32, tag="pst")
                        sums = p2sb.tile([128, NCH], F32, tag="sums")
                        for c in range(nchunk):
                            n = min(512, ncols - c * 512)
                            pps = p512.tile([128, 512], F32, tag="p512", name=f"pps{h}_{st}_{c}")
                            nc.tensor.matmul(
                                pps[:, 0:n], qT[h][:, s0 : s0 + 128],
                                kT[h][:, c * 512 : c * 512 + n],
                                start=True, stop=True,
                            )
                            if causal and c == nchunk - 1:
                                off = ncols - 128 - c * 512
                                nc.vector.tensor_add(
                                    pps[:, off : off + 128], pps[:, off : off + 128], mkd[:]
                                )
                            elif mask_mode == "full":
                                mt = mpool.tile([128, 512], F32, tag="mst")
                                nc.sync.dma_start(mt[:, 0:n], mk_d[s0 : s0 + 128, c * 512 : c * 512 + n])
                                nc.vector.tensor_add(pps[:, 0:n], pps[:, 0:n], mt[:, 0:n])
                            nc.scalar.activation(
                                pst[:, c * 512 : c * 512 + n], pps[:, 0:n], EXP,
                                scale=SCALE, accum_out=sums[:, c : c + 1],
                            )
                        tot = p2sb.tile([128, 1], F32, tag="tot")
                        if nchunk > 1:
                            nc.vector.reduce_sum(tot[:], sums[:, 0:nchunk], axis=mybir.AxisListType.X)
                        else:
                            nc.vector.tensor_copy(tot[:], sums[:, 0:1])
                        nc.vector.reciprocal(recip_all[h][:, st : st + 1], tot[:])
                        pno = p2sb3.tile([128, S], F32, tag="pno")
                        nc.vector.tensor_scalar_mul(
                            pno[:, 0:ncols], pst[:, 0:ncols], recip_all[h][:, st : st + 1]
                        )
                        nc.sync.dma_start(aw_d[h, s0 : s0 + 128, 0:ncols], pno[:, 0:ncols])

                    # ---- T orientation + AV: chunk-pairs, software-pipelined ----
                    tmax_of = (lambda c: c * 4 + 3) if causal else (lambda c: NT - 1)
                    cps = {}
                    for pair in ((0, 1), (2, 3)):
                        for c in pair:
                            cps[c] = ctxps.tile(
                                [128, 512], F32, tag=f"ctx{c}", name=f"ctx{h}_{c}"
                            )
                        nts = max(tmax_of(c) for c in pair) + 1
                        pending = None  # (t, {c: (off, ptsb)})
                        for t in range(nts):
                            t0 = t * 128
                            chunks = [c for c in pair if (not causal) or t <= tmax_of(c)]
                            cur = {}
                            for c in chunks:
                                off = max(0, t0 - c * 512) if causal else 0
                                ptp = p512.tile([128, 512], F32, tag="p512", name=f"ptp{h}_{t}_{c}")
                                nc.tensor.matmul(
                                    ptp[:, off:512], kT[h][:, t0 : t0 + 128],
                                    qT[h][:, c * 512 + off : (c + 1) * 512],
                                    start=True, stop=True,
                                )
                                cur[c] = (off, ptp)
                            if pending is not None:
                                pt, pcur = pending
                                for c, (off, ptsb) in pcur.items():
                                    nc.tensor.matmul(
                                        cps[c][:, off:512], v_all[:, pt, h * D : (h + 1) * D],
                                        ptsb[:, off:512],
                                        start=(pt == 0), stop=(pt == tmax_of(c)),
                                        skip_group_check=True,
                                    )
                                pending = None
                            for c in chunks:
                                off, ptp = cur[c]
                                if causal and t // 4 == c:
                                    nc.vector.tensor_add(
                                        ptp[:, off : off + 128], ptp[:, off : off + 128], mkdT[:]
                                    )
                                elif mask_mode == "full":
                                    mtT = mpool.tile([128, 512], F32, tag="mtT")
                                    nc.sync.dma_start(mtT[:], mkT_d[t0 : t0 + 128, c * 512 : (c + 1) * 512])
                                    nc.vector.tensor_add(ptp[:], ptp[:], mtT[:])
                                ptsb = ptpool.tile([128, 512], F32R, tag="ptsb", name=f"ptsb{h}_{t}_{c}")
                                nc.scalar.activation(ptsb[:, off:512], ptp[:, off:512], EXP, scale=SCALE)
                                cur[c] = (off, ptsb)
                            pending = (t, cur)
                        pt, pcur = pending
                        for c, (off, ptsb) in pcur.items():
                            nc.tensor.matmul(
                                cps[c][:, off:512], v_all[:, pt, h * D : (h + 1) * D],
                                ptsb[:, off:512],
                                start=(pt == 0), stop=(pt == tmax_of(c)),
                                skip_group_check=True,
                            )

                    # ---- row-layout recip broadcast:
                    # rbc[p, s] = 1/rowsum[s], via PE transpose + select-matmuls
                    rt_ps = p512.tile([16, 128], F32, tag="p512", name=f"rtps{h}")
                    nc.tensor.transpose(rt_ps[:], recip_all[h][:], ident[:])
                    rt16 = p2sb.tile([16, 128], F32R, tag="rt16")
                    nc.vector.tensor_copy(rt16[:], rt_ps[:])
                    rbc = p2sb.tile([128, S], F32, tag="rbc")
                    for st in range(NT):
                        rb_ps = p512.tile([128, 128], F32, tag="p512", name=f"rbps{h}_{st}")
                        nc.tensor.matmul(
                            rb_ps[:], sel_sb[:, st * 128 : (st + 1) * 128], rt16[:],
                            start=True, stop=True,
                        )
                        nc.vector.tensor_copy(rbc[:, st * 128 : (st + 1) * 128], rb_ps[:])
                    for c in range(NCH):
                        nc.vector.tensor_mul(
                            ctxT[h][:, c * 512 : (c + 1) * 512], cps[c][:],
                            rbc[:, c * 512 : (c + 1) * 512],
                        )

            # ================= Phase 3: out projection (transposed) =================
            # outp[hid, s] = sum_d woutT[d, hid] * ctxT[d, s], accumulated over heads
            with ExitStack() as p3:
                ops_ = p3.enter_context(tc.tile_pool(name="ops", bufs=2, space="PSUM"))
                osb_p = p3.enter_context(tc.tile_pool(name="osbp", bufs=2))
                for ht in range(NT):
                    hh0 = ht * 128
                    osb = osb_p.tile([128, S], F32, tag="osb")
                    o_ps = [
                        ops_.tile([128, 512], F32, tag=f"o{c}", name=f"o{ht}_{c}")
                        for c in range(NCH)
                    ]
                    for h in range(HPC):
                        for c in range(NCH):
                            nc.tensor.matmul(
                                o_ps[c][:], wo_sb[:, h, hh0 : hh0 + 128],
                                ctxT[h][:, c * 512 : (c + 1) * 512],
                                start=(h == 0), stop=(h == HPC - 1),
                                skip_group_check=True,
                            )
                    for c in range(NCH):
                        nc.any.tensor_copy(osb[:, c * 512 : (c + 1) * 512], o_ps[c][:])
                    nc.sync.dma_start(op_d[hh0 : hh0 + 128, :], osb[:])

    nc.compile()
    return nc


def _host_consts():
    inv_freq = 1.0 / (10000.0 ** (np.arange(0, D, 2, dtype=np.float64) / D))  # [64]
    ang = np.arange(S, dtype=np.float64)[:, None] * inv_freq[None, :]          # [S, 64]
    cos = np.cos(ang).astype(np.float32)
    sin = np.sin(ang).astype(np.float32)
    cosrep = np.tile(cos, (1, 4))                                              # [S, 256]
    sinsgn = np.concatenate([-sin, sin, -sin, sin], axis=1)                    # [S, 256]
    i = np.arange(128)
    maskd = np.where(i[:, None] >= i[None, :], 0.0, NEG_INF).astype(np.float32)
    maskdT = maskd.T.copy()
    selmat = np.zeros((NT, NT * 128), dtype=np.float32)
    for st in range(NT):
        selmat[st, st * 128 : (st + 1) * 128] = 1.0
    return cosrep, sinsgn, maskd, maskdT, selmat


def _detect_mode(attn_mask):
    if not np.any(attn_mask):
        return "none"
    i = np.arange(S)
    causal_ref = np.where(i[:, None] >= i[None, :], 0.0, np.float32(NEG_INF)).astype(np.float32)
    if np.array_equal(attn_mask, causal_ref):
        return "causal"
    return "full"


def kernel(hidden_states, attn_mask, w_qkv, w_out):
    hidden_states = np.ascontiguousarray(hidden_states, dtype=np.float32)
    attn_mask = np.ascontiguousarray(attn_mask, dtype=np.float32)
    w_qkv = np.ascontiguousarray(w_qkv, dtype=np.float32)
    w_out = np.ascontiguousarray(w_out, dtype=np.float32)

    mode = _detect_mode(attn_mask)
    if mode not in _BUILD_CACHE:
        _BUILD_CACHE[mode] = build(mode)
    nc = _BUILD_CACHE[mode]

    cosrep, sinsgn, maskd, maskdT, selmat = _host_consts()
    hT = np.ascontiguousarray(hidden_states[0].T)          # [HIDDEN, S]

    in_maps = []
    for core in range(N_CORES):
        heads = [HPC * core + j for j in range(HPC)]
        d = np.arange(D)
        rows = np.concatenate(
            [base + d * HEADS + h for base in (0, QKV, 2 * QKV) for h in heads]
        )
        wqkvT = np.ascontiguousarray(w_qkv[rows, :].T)     # [HIDDEN, 768]
        cols = np.concatenate([d * HEADS + h for h in heads])
        woutT = np.ascontiguousarray(w_out[:, cols].T)     # [256, HIDDEN]
        m = {
            "hT": hT, "wqkvT": wqkvT, "woutT": woutT,
            "cosrep": cosrep, "sinsgn": sinsgn, "selmat": selmat,
        }
        if mode == "causal":
            m["maskd"] = maskd
            m["maskdT"] = maskdT
        if mode == "full":
            m["mask"] = attn_mask
            m["maskT"] = np.ascontiguousarray(attn_mask.T)
        in_maps.append(m)

    trace = bool(int(os.environ.get("KERNEL_TRACE", "0")))
    kwargs = {}
    if trace:
        import ntff_shim
        ntff_shim.install()
        kwargs = {"trace": True, "trace_cores": [0]}
    res = bass_utils.run_bass_kernel_spmd(nc, in_maps, core_ids=list(range(N_CORES)), **kwargs)
    kernel.last_exec_time_ns = res.exec_time_ns
    kernel.last_results = res

    attn_output_T = np.zeros((HIDDEN, S), dtype=np.float32)
    aw = np.empty((HEADS, S, S), dtype=np.float32)
    pk = np.empty((HEADS, S, D), dtype=np.float32)
    pv = np.empty((HEADS, S, D), dtype=np.float32)
    for core in range(N_CORES):
        r = res.results[core]
        attn_output_T += r["outp"]
        aw[HPC * core : HPC * (core + 1)] = r["aw"]
        pk[HPC * core : HPC * (core + 1)] = r["pk"].reshape(S, HPC, D).transpose(1, 0, 2)
        pv[HPC * core : HPC * (core + 1)] = r["pv"].reshape(S, HPC, D).transpose(1, 0, 2)

    return (
        np.ascontiguousarray(attn_output_T.T)[None, :, :],
        aw[None, :, :, :],
        pk[None, :, :, :],
        pv[None, :, :, :],
    )
